# revision 61
# baseline (speedup 1.0000x reference)
"""Distributed multi-head attention kernel for 8 TRN2 NeuronCores.

Problem: B=2, N=2048, C=1024, H=16 heads, D=64.
  out = softmax((q@Wq)(k@Wk)^T / sqrt(D)) @ (v@Wv) @ Wo   (per head, biases zero)

Sharding: batch x head-group.  Core c owns batch b=c//4 and head group
g=c%4 -> heads [4g, 4g+4) = channel block [256g, 256g+256).
Zero-redundancy: each core projects only its own 256 Q/K/V channels for
its batch, runs attention for its 4 heads over all 2048 queries/keys,
and computes the row-sharded out-proj partial out^T = Wo_s^T @ A^T
(bf16).  The host sums the 4 partials per batch (the "all-reduce" of
the sharding hint, done at gather time) -- no device collectives.

Per-core engine budget (measured): PE ~210us active (matmul columns
164us @2.4GHz -- scores and PV are stream-bound at D=64 -- plus
ldweights/p-state tax), ScalarE 142.6us of exp (128 x [128,1024]
ACTIVATE @ ~1114ns), ~14MB input DMA.  The kernel is PE-bound, so the
schedule keeps the PE streaming from ~12us on and ScalarE as close
behind as the data deadlines allow:

  - inputs arrive as batched 3D-AP DMAs (512-column quarters for
    xq/xk) split across both hardware DGE queues (SP: q/v-side + wo;
    Activation: k-side) so the ~3MB the first score pairs need lands
    first; one long junk-matmul ACCUMULATION group (no per-matmul
    semaphores) warms the PE clock during the wait.
  - query-block-major pipeline with one "slot" per score group (one
    exp, ~1.1us).  Per 512-query block qb, 4 heads x 8 score pairs
    stream into ScalarE; PV of lagging heads, out-proj of qb-1, and
    the Q projection of qb+1 ride pair-by-pair inside the slots.
    The last head's PV is slot-lagged by one pair so only one pair +
    normalize + out-proj remain after the final exp.
  - PV is V'-stationary (65 weight cols; col 64 = ones accumulates
    the softmax denominator in psum row 64).  A P-stationary PV costs
    +214us in serial 128-col weight reloads -- measured.
  - normalize: copy denom row -> reciprocal -> gpsimd partition
    broadcast -> multiply, writing A^T planes directly (no transposes).
  - SBUF: x tensors live on the right-side allocator stack, released
    LIFO (xk -> xv -> xq) so the 50KB P pool fits.
  - PSUM: 4 banks of score groups (2x[128,1024]) + 2 PV banks +
    2 shared banks for qk/v/out-proj groups = exactly 8.

Measured 244-246us (vs 296.9us baseline) on a cold device; the
device clocks throttle ~1.2x under repeated back-to-back runs.
"""

import sys

sys.path.insert(0, "/opt/trn_rl_repo")

from contextlib import ExitStack

import numpy as np
import ml_dtypes

import concourse.bass as bass
import concourse.bacc as bacc
import concourse.mybir as mybir
import concourse.tile as tile
from concourse.bass_utils import run_bass_kernel_spmd

BF16 = mybir.dt.bfloat16
F32 = mybir.dt.float32
Exp = mybir.ActivationFunctionType.Exp

B, N, C = 2, 2048, 1024
H, D = 16, 64
HC = 4              # heads per core
CB = HC * D         # own channel block = 256
DV = D + 1          # V cols per head incl. ones column
NCHUNK = N // 128   # 16 key chunks
NQB = N // 512      # 4 query blocks
SCALE = 1.0 / np.sqrt(D)

_CACHE = {}


def build_nc():
    nc = bacc.Bacc("TRN2", target_bir_lowering=False, debug=False, num_devices=8)

    xqT = nc.declare_dram_parameter("xqT", [C, N], BF16, isOutput=False)
    xkT = nc.declare_dram_parameter("xkT", [C, N], BF16, isOutput=False)
    xvT = nc.declare_dram_parameter("xvT", [C, N], BF16, isOutput=False)
    wq = nc.declare_dram_parameter("wq", [C, CB], BF16, isOutput=False)
    wk = nc.declare_dram_parameter("wk", [C, CB], BF16, isOutput=False)
    wv = nc.declare_dram_parameter("wv", [C, CB], BF16, isOutput=False)
    wo = nc.declare_dram_parameter("wo", [CB, C], BF16, isOutput=False)
    outT = nc.declare_dram_parameter("outT", [C, N], BF16, isOutput=True)

    with tile.TileContext(nc) as tc, ExitStack() as top:
        # ---------------- resident SBUF ----------------
        res = top.enter_context(tc.tile_pool(name="res", bufs=1))
        # Q^T / K^T: plane p holds head 2p in rows 0:64, head 2p+1 in 64:128
        qT_sb = res.tile([128, 2 * N], BF16, tag="qT")
        kT_sb = res.tile([128, 2 * N], BF16, tag="kT")
        # V' is 65 cols per (kc, h): col 64 = ones so the PV matmul
        # accumulates the softmax denominator in psum row 64.
        v1_sb = res.tile([128, NCHUNK * HC * DV], BF16, tag="v1")
        aT0_sb = res.tile([128, N], BF16, tag="aT0")   # A^T rows 0:128 (h 0,1)
        aT1_sb = res.tile([128, N], BF16, tag="aT1")   # A^T rows 128:256 (h 2,3)
        draw_sb = res.tile([1, 512], F32, tag="draw")
        drow_sb = res.tile([1, 512], F32, tag="drow")

        def q_slice(h, qb):
            base = N * (h // 2)
            return qT_sb[64 * (h % 2):64 * (h % 2) + 64,
                         base + 512 * qb:base + 512 * (qb + 1)]

        def k_slice(h, kc):
            base = N * (h // 2)
            return kT_sb[64 * (h % 2):64 * (h % 2) + 64,
                         base + 128 * kc:base + 128 * (kc + 1)]

        v3 = v1_sb[:].rearrange("p (kc h x) -> p kc h x", kc=NCHUNK, x=DV)

        # ---------------- pools ----------------
        main = ExitStack()
        wpool = main.enter_context(tc.tile_pool(name="wpool", bufs=4))
        P_pool = main.enter_context(tc.tile_pool(name="P_pool", bufs=25))
        dpool = main.enter_context(tc.tile_pool(name="dpool", bufs=2))
        ospool = main.enter_context(tc.tile_pool(name="ospool", bufs=3))
        spool = main.enter_context(
            tc.tile_pool(name="spool", bufs=2, space="PSUM"))   # 2x2 banks
        pvpool = main.enter_context(
            tc.tile_pool(name="pvpool", bufs=2, space="PSUM"))  # 2x1 banks
        gpool = main.enter_context(
            tc.tile_pool(name="gpool", bufs=2, space="PSUM"))   # 2x1 banks
        xq_stack = ExitStack()
        xqpool = xq_stack.enter_context(
            tc.tile_pool(name="xqpool", bufs=4, side="right"))
        xv_stack = ExitStack()
        xvpool = xv_stack.enter_context(
            tc.tile_pool(name="xvpool", bufs=2, side="right"))
        xk_stack = ExitStack()
        xkpool = xk_stack.enter_context(
            tc.tile_pool(name="xkpool", bufs=4, side="right"))

        # -------- input DMA: one batched transfer per half-tensor --------
        # DRAM [1024, n] viewed as [128 partitions, 8 cc-chunks, n].
        def dram3(t, lo, hi):
            return t[:].rearrange("(c p) n -> p c n", p=128)[:, :, lo:hi]

        wq_t = res.tile([128, 8 * CB], BF16, tag="wqt")
        wk_t = res.tile([128, 8 * CB], BF16, tag="wkt")
        wv_t = res.tile([128, 8 * CB], BF16, tag="wvt")
        wo_t = res.tile([128, 2 * C], BF16, tag="wot")
        wq3 = wq_t[:].rearrange("p (c n) -> p c n", c=8)
        wk3 = wk_t[:].rearrange("p (c n) -> p c n", c=8)
        wv3 = wv_t[:].rearrange("p (c n) -> p c n", c=8)
        wo3 = wo_t[:].rearrange("p (j n) -> p j n", j=2)

        xq_t = [xqpool.tile([128, 8 * 512], BF16, tag="xq", name=f"xq{i}")
                for i in range(4)]
        xk_t = [xkpool.tile([128, 8 * 512], BF16, tag="xk", name=f"xk{i}")
                for i in range(4)]
        xv_t = [xvpool.tile([128, 8 * 1024], BF16, tag="xv", name=f"xv{i}")
                for i in range(2)]
        xq3 = [t[:].rearrange("p (c n) -> p c n", c=8) for t in xq_t]
        xk3 = [t[:].rearrange("p (c n) -> p c n", c=8) for t in xk_t]
        xv3 = [t[:].rearrange("p (c n) -> p c n", c=8) for t in xv_t]

        def xq_sl(cc, qb):
            return xq3[qb][:, cc, :]

        def xk_sl(cc, kb):
            return xk3[kb][:, cc, :]

        nc.sync.dma_start(out=wq3[:], in_=dram3(wq, 0, CB))
        nc.scalar.dma_start(out=wk3[:], in_=dram3(wk, 0, CB))
        for i in range(4):
            nc.sync.dma_start(out=xq3[i][:], in_=dram3(xqT, 512 * i, 512 * (i + 1)))
            nc.scalar.dma_start(out=xk3[i][:], in_=dram3(xkT, 512 * i, 512 * (i + 1)))
        nc.sync.dma_start(out=wv3[:], in_=dram3(wv, 0, CB))
        for i in range(2):
            nc.sync.dma_start(out=xv3[i][:], in_=dram3(xvT, 1024 * i, 1024 * (i + 1)))
        nc.sync.dma_start(out=wo3[:],
                          in_=wo[:].rearrange("(j p) n -> p j n", p=128))

        nc.vector.memset(v3[:, :, :, D:DV], 1.0)

        # Warm the PE p-state during the input-DMA wait.  One long
        # ACCUMULATION group (start only on the first matmul) so the
        # junk matmuls stream without per-instruction semaphore chains.
        jk = gpool.tile([128, 512], F32, tag="g", name="junk")
        NJUNK = 24
        for i in range(NJUNK):
            nc.tensor.matmul(jk[:], v1_sb[:, 0:128], v1_sb[:, 0:512],
                             start=(i == 0), stop=(i == NJUNK - 1))

        # ---------------- building blocks ----------------
        P_tiles, PV, qk_state = {}, {}, {}

        def scores_pair(h, qb, pair):
            """S^T + exp for chunks (2*pair, 2*pair+1) of head h, qblock qb."""
            st = spool.tile([128, 1024], F32, tag="st", name=f"st_{h}_{qb}_{pair}")
            Pp = P_pool.tile([128, 1024], BF16, tag="P", name=f"P_{h}_{qb}_{pair}")
            for i in range(2):
                kc = 2 * pair + i
                nc.tensor.matmul(st[:, 512 * i:512 * (i + 1)],
                                 k_slice(h, kc), q_slice(h, qb),
                                 start=True, stop=True)
            nc.scalar.activation(Pp[:], st[:], Exp, scale=float(SCALE))
            P_tiles[(h, qb, pair)] = Pp

        def qk_proj_part(w3, x_sl, dst_sb, mb, qb, part, nparts):
            """1/nparts of one [128,512] Q^T/K^T projection group."""
            key = (id(w3), mb, qb)
            if part == 0:
                qk_state[key] = gpool.tile([128, 512], F32, tag="g",
                                           name=f"qk{mb}_{qb}_{id(w3) % 97}")
            ps = qk_state[key]
            step = 8 // nparts
            for cc in range(step * part, step * (part + 1)):
                nc.tensor.matmul(ps[:],
                                 w3[:, cc, 128 * mb:128 * (mb + 1)],
                                 x_sl(cc, qb),
                                 start=(cc == 0), stop=(cc == 7))
            if part == nparts - 1:
                nc.vector.tensor_copy(
                    dst_sb[:, N * mb + 512 * qb:N * mb + 512 * (qb + 1)], ps[:])
                del qk_state[key]

        def v_proj_block(tb):
            """V' for key-chunk tb: out[128 keys, 256] -> v1 cols 0:64."""
            ps = gpool.tile([128, 512], F32, tag="g", name=f"vps{tb}")
            for cc in range(8):
                nc.tensor.matmul(ps[:, 0:CB],
                                 xv3[tb // 8][:, cc, 128 * (tb % 8):
                                              128 * (tb % 8) + 128],
                                 wv3[:, cc, :],
                                 start=(cc == 0), stop=(cc == 7))
            nc.vector.tensor_copy(
                v3[:, tb, :, 0:D],
                ps[:, 0:CB].rearrange("p (h d) -> p h d", d=D))

        def pv_part(h, qb, pair):
            """Two PV chunk-matmuls for head h / qblock qb; finishes at pair 7.

            po rows 0:64 = O^T(h) raw, row 64 = softmax denominator.
            """
            if pair == 0:
                PV[(h, qb)] = pvpool.tile([128, 512], F32, tag="po",
                                          name=f"po{h}_{qb}")
            po = PV[(h, qb)]
            Pp = P_tiles.pop((h, qb, pair))
            for i in range(2):
                kc = 2 * pair + i
                nc.tensor.matmul(po[0:DV, :],
                                 v3[:, kc, h, :],
                                 Pp[:, 512 * i:512 * (i + 1)],
                                 start=(kc == 0), stop=(kc == NCHUNK - 1))
            if pair == 7:
                pv_finish(h, qb)

        def pv_finish(h, qb):
            """Normalize: A^T(h) = po[0:64] / po[64] -> aT plane."""
            po = PV.pop((h, qb))
            dinv = dpool.tile([64, 512], F32, tag="dinv", name=f"di{h}_{qb}")
            nc.vector.tensor_copy(draw_sb[:], po[64:65, :])
            nc.vector.reciprocal_approx_fast(drow_sb[:], draw_sb[:])
            nc.gpsimd.partition_broadcast(dinv[:], drow_sb[:])
            dst = aT0_sb if h < 2 else aT1_sb
            nc.vector.tensor_mul(
                dst[64 * (h % 2):64 * (h % 2) + 64, 512 * qb:512 * (qb + 1)],
                po[0:D, :], dinv[:])

        def oproj_m(qb, m, scalar_cast=False):
            """One m-block of the out-proj partial for query block qb.

            scalar_cast routes the psum->sbuf cast to ScalarE (a Copy,
            resident in every activation table set) -- used after the
            final exp, when ScalarE is idle, to unserialize the tail.
            """
            ps = gpool.tile([128, 512], F32, tag="g", name=f"ops{m}_{qb}")
            for j in range(2):
                aT = (aT0_sb, aT1_sb)[j]
                nc.tensor.matmul(ps[:], wo3[:, j, 128 * m:128 * (m + 1)],
                                 aT[:, 512 * qb:512 * (qb + 1)],
                                 start=(j == 0), stop=(j == 1))
            ev = ospool.tile([128, 512], BF16, tag="ev", name=f"oev{m}_{qb}")
            if scalar_cast:
                nc.scalar.copy(ev[:], ps[:])
            else:
                nc.vector.tensor_copy(ev[:], ps[:])
            nc.sync.dma_start(
                out=outT[128 * m:128 * (m + 1), 512 * qb:512 * (qb + 1)],
                in_=ev[:])

        # ---------------- emission ----------------
        # Pre-loop: the FIRST score pair fires right after the minimal
        # projections it needs (Q^T plane 0 of qb0 + K^T plane-0 block
        # 0); the rest of the warmup projections follow, and K blocks
        # 2/3 ride as slot items ahead of the pairs that need them.
        qk_proj_part(wq3, xq_sl, qT_sb, 0, 0, 0, 1)
        qk_proj_part(wk3, xk_sl, kT_sb, 0, 0, 0, 1)
        scores_pair(0, 0, 0)
        qk_proj_part(wq3, xq_sl, qT_sb, 1, 0, 0, 1)
        qk_proj_part(wk3, xk_sl, kT_sb, 0, 1, 0, 1)

        # Slot schedule: one slot = one score group = one exp (~1.1us);
        # each slot carries <=~1.3us of extra PE work.
        #   qb0 h0: K^T plane-1 half-groups    (needed by h2 scores)
        #       h1: V' chunk 2p
        #       h2: V' chunk 2p+1 (pre), pv(h0) pair p
        #       h3: pv(h1) p, pv(h2) p-1, Q^T(qb1) quarters on p<4
        #       tail: pv(h2) pair 7        [pv(h3) rides in qb1's slots]
        #   qb>=1: pv(h-1) in-slot; fillers from the item list below;
        #       h3 additionally slot-lags pv(h3) by one pair, tail = pair 7.
        def run_qblock(qb, pre_items, post_items):
            lag = 2 if qb == 0 else 1
            for h in range(HC):
                for pair in range(8):
                    if not (qb == 0 and h == 0 and pair == 0):
                        scores_pair(h, qb, pair)
                    for it in pre_items.get((h, pair), ()):
                        it()
                    if h >= lag:
                        pv_part(h - lag, qb, pair)
                    if h == HC - 1 and lag == 1 and pair >= 1:
                        pv_part(HC - 1, qb, pair - 1)
                    for it in post_items.get((h, pair), ()):
                        it()
            if qb == 0:
                pv_part(HC - 2, 0, 7)
            else:
                pv_part(HC - 1, qb, 7)

        pre0, post0 = {}, {}
        # K plane-0 blocks 2/3 ride the earliest h0 slots (well before
        # score pairs 4 and 6 need them); the displaced K plane-1
        # halves slide into h1 slots.
        post0[(0, 0)] = [lambda: qk_proj_part(wk3, xk_sl, kT_sb, 0, 2, 0, 1)]
        post0[(0, 1)] = [lambda: qk_proj_part(wk3, xk_sl, kT_sb, 0, 3, 0, 1)]
        km1 = [lambda kb=kb, part=part:
               qk_proj_part(wk3, xk_sl, kT_sb, 1, kb, part, 2)
               for kb in range(NQB) for part in range(2)]
        for p in (2, 3, 4, 5, 6, 7):
            post0[(0, p)] = [km1.pop(0)]
        for p in range(8):
            post0[(1, p)] = ([km1.pop(0)] if km1 else []) + \
                [lambda tb=2 * p: v_proj_block(tb)]
            pre0[(2, p)] = [lambda tb=2 * p + 1: v_proj_block(tb)]
            post0[(3, p)] = [lambda pp=p - 1: pv_part(2, 0, pp)] if p >= 1 else []
            if p < 4:
                post0[(3, p)] = post0.get((3, p), []) + \
                    [lambda mb=p // 2, part=p % 2:
                     qk_proj_part(wq3, xq_sl, qT_sb, mb, 1, part, 2)]
        run_qblock(0, pre0, post0)
        xk_stack.close()
        xv_stack.close()

        for qb in range(1, NQB):
            items = []
            if qb == 1:
                items += [lambda p=p: pv_part(3, 0, p) for p in range(8)]
            opq = [lambda m=m, q=qb - 1: oproj_m(q, m) for m in range(8)]
            if qb < NQB - 1:
                qqs = [lambda mb=mb, part=part, q=qb + 1:
                       qk_proj_part(wq3, xq_sl, qT_sb, mb, q, part, 4)
                       for mb in range(2) for part in range(4)]
                inter = [x for pair in zip(opq, qqs) for x in pair]
            else:
                inter = opq
            items += inter
            sched = {}
            for s, it in enumerate(items):
                sched[(s // 8, s % 8)] = sched.get((s // 8, s % 8), []) + [it]
            run_qblock(qb, {}, sched)
            if qb == NQB - 1:
                # keep the PE clock warm through the normalize chain so
                # the tail out-proj runs at full speed
                jk2 = gpool.tile([128, 512], F32, tag="g", name="junk2")
                for i in range(8):
                    nc.tensor.matmul(jk2[:], v1_sb[:, 0:128], v1_sb[:, 0:512],
                                     start=(i == 0), stop=(i == 7))
            if qb == NQB - 2:
                xq_stack.close()
        pv_finish(HC - 1, NQB - 1) if (HC - 1, NQB - 1) in PV else None
        for m in range(8):
            oproj_m(NQB - 1, m, scalar_cast=(m % 2 == 1))
        main.close()

    nc.compile()
    return nc


def _get_nc():
    if "nc" not in _CACHE:
        _CACHE["nc"] = build_nc()
    return _CACHE["nc"]


def _make_in_maps(q, k, v, Wq, Wk, Wv, Wo):
    bf = ml_dtypes.bfloat16
    q, k, v = np.asarray(q), np.asarray(k), np.asarray(v)
    qT = [np.ascontiguousarray(q[b].T).astype(bf) for b in range(B)]
    kT = [np.ascontiguousarray(k[b].T).astype(bf) for b in range(B)]
    vT = [np.ascontiguousarray(v[b].T).astype(bf) for b in range(B)]
    Wq, Wk, Wv, Wo = (np.asarray(x) for x in (Wq, Wk, Wv, Wo))
    wq_s = [np.ascontiguousarray(Wq[:, CB * g:CB * (g + 1)]).astype(bf)
            for g in range(4)]
    wk_s = [np.ascontiguousarray(Wk[:, CB * g:CB * (g + 1)]).astype(bf)
            for g in range(4)]
    wv_s = [np.ascontiguousarray(Wv[:, CB * g:CB * (g + 1)]).astype(bf)
            for g in range(4)]
    wo_s = [np.ascontiguousarray(Wo[CB * g:CB * (g + 1), :]).astype(bf)
            for g in range(4)]
    in_maps = []
    for c in range(8):
        b, g = c // 4, c % 4
        in_maps.append({
            "xqT": qT[b], "xkT": kT[b], "xvT": vT[b],
            "wq": wq_s[g], "wk": wk_s[g], "wv": wv_s[g], "wo": wo_s[g],
        })
    return in_maps


def _run(inputs, trace=False, **kw):
    nc = _get_nc()
    in_maps = _make_in_maps(inputs["q"], inputs["k"], inputs["v"],
                            inputs["Wq"], inputs["Wk"], inputs["Wv"], inputs["Wo"])
    res = None
    for attempt in range(3):
        try:
            res = run_bass_kernel_spmd(nc, in_maps, core_ids=list(range(8)),
                                       trace=trace, **kw)
            break
        except Exception:
            if attempt == 2:
                raise
            import time
            time.sleep(2.0)
    out = np.empty((B, N, C), np.float32)
    for b in range(B):
        acc = np.zeros((C, N), np.float32)
        for g in range(4):
            acc += res.results[4 * b + g]["outT"].astype(np.float32)
        out[b] = acc.T
    return out, res


def kernel(**inputs) -> np.ndarray:
    out, _ = _run(inputs, trace=False)
    return out


# revision 62
# speedup vs baseline: 1.0006x; 1.0006x over previous
"""Distributed multi-head attention kernel for 8 TRN2 NeuronCores.

Problem: B=2, N=2048, C=1024, H=16 heads, D=64.
  out = softmax((q@Wq)(k@Wk)^T / sqrt(D)) @ (v@Wv) @ Wo   (per head, biases zero)

Sharding: batch x head-group.  Core c owns batch b=c//4 and head group
g=c%4 -> heads [4g, 4g+4) = channel block [256g, 256g+256).
Zero-redundancy: each core projects only its own 256 Q/K/V channels for
its batch, runs attention for its 4 heads over all 2048 queries/keys,
and computes the row-sharded out-proj partial out^T = Wo_s^T @ A^T
(bf16).  The host sums the 4 partials per batch (the "all-reduce" of
the sharding hint, done at gather time) -- no device collectives.

Per-core engine budget (measured): PE ~210us active (matmul columns
164us @2.4GHz -- scores and PV are stream-bound at D=64 -- plus
ldweights/p-state tax), ScalarE 142.6us of exp (128 x [128,1024]
ACTIVATE @ ~1114ns), ~14MB input DMA.  The kernel is PE-bound, so the
schedule keeps the PE streaming from ~12us on and ScalarE as close
behind as the data deadlines allow:

  - inputs arrive as batched 3D-AP DMAs (512-column quarters for
    xq/xk) split across both hardware DGE queues (SP: q/v-side + wo;
    Activation: k-side) so the ~3MB the first score pairs need lands
    first; one long junk-matmul ACCUMULATION group (no per-matmul
    semaphores) warms the PE clock during the wait.
  - query-block-major pipeline with one "slot" per score group (one
    exp, ~1.1us).  Per 512-query block qb, 4 heads x 8 score pairs
    stream into ScalarE; PV of lagging heads, out-proj of qb-1, and
    the Q projection of qb+1 ride pair-by-pair inside the slots.
    The last head's PV is slot-lagged by one pair so only one pair +
    normalize + out-proj remain after the final exp.
  - PV is V'-stationary (65 weight cols; col 64 = ones accumulates
    the softmax denominator in psum row 64).  A P-stationary PV costs
    +214us in serial 128-col weight reloads -- measured.
  - normalize: copy denom row -> reciprocal -> gpsimd partition
    broadcast -> multiply, writing A^T planes directly (no transposes).
  - SBUF: x tensors live on the right-side allocator stack, released
    LIFO (xk -> xv -> xq) so the 50KB P pool fits.
  - PSUM: 4 banks of score groups (2x[128,1024]) + 2 PV banks +
    2 shared banks for qk/v/out-proj groups = exactly 8.

Measured 244-246us (vs 296.9us baseline) on a cold device; the
device clocks throttle ~1.2x under repeated back-to-back runs.
"""

import sys

sys.path.insert(0, "/opt/trn_rl_repo")

from contextlib import ExitStack

import numpy as np
import ml_dtypes

import concourse.bass as bass
import concourse.bacc as bacc
import concourse.mybir as mybir
import concourse.tile as tile
from concourse.bass_utils import run_bass_kernel_spmd

BF16 = mybir.dt.bfloat16
F32 = mybir.dt.float32
Exp = mybir.ActivationFunctionType.Exp

B, N, C = 2, 2048, 1024
H, D = 16, 64
HC = 4              # heads per core
CB = HC * D         # own channel block = 256
DV = D + 1          # V cols per head incl. ones column
NCHUNK = N // 128   # 16 key chunks
NQB = N // 512      # 4 query blocks
SCALE = 1.0 / np.sqrt(D)

_CACHE = {}


def build_nc():
    nc = bacc.Bacc("TRN2", target_bir_lowering=False, debug=False, num_devices=8)

    xqT = nc.declare_dram_parameter("xqT", [C, N], BF16, isOutput=False)
    xkT = nc.declare_dram_parameter("xkT", [C, N], BF16, isOutput=False)
    xvT = nc.declare_dram_parameter("xvT", [C, N], BF16, isOutput=False)
    wq = nc.declare_dram_parameter("wq", [C, CB], BF16, isOutput=False)
    wk = nc.declare_dram_parameter("wk", [C, CB], BF16, isOutput=False)
    wv = nc.declare_dram_parameter("wv", [C, CB], BF16, isOutput=False)
    wo = nc.declare_dram_parameter("wo", [CB, C], BF16, isOutput=False)
    outT = nc.declare_dram_parameter("outT", [C, N], BF16, isOutput=True)

    with tile.TileContext(nc) as tc, ExitStack() as top:
        # ---------------- resident SBUF ----------------
        res = top.enter_context(tc.tile_pool(name="res", bufs=1))
        # Q^T / K^T: plane p holds head 2p in rows 0:64, head 2p+1 in 64:128
        qT_sb = res.tile([128, 2 * N], BF16, tag="qT")
        kT_sb = res.tile([128, 2 * N], BF16, tag="kT")
        # V' is 65 cols per (kc, h): col 64 = ones so the PV matmul
        # accumulates the softmax denominator in psum row 64.
        v1_sb = res.tile([128, NCHUNK * HC * DV], BF16, tag="v1")
        aT0_sb = res.tile([128, N], BF16, tag="aT0")   # A^T rows 0:128 (h 0,1)
        aT1_sb = res.tile([128, N], BF16, tag="aT1")   # A^T rows 128:256 (h 2,3)
        draw_sb = res.tile([1, 512], F32, tag="draw")
        drow_sb = res.tile([1, 512], F32, tag="drow")

        def q_slice(h, qb):
            base = N * (h // 2)
            return qT_sb[64 * (h % 2):64 * (h % 2) + 64,
                         base + 512 * qb:base + 512 * (qb + 1)]

        def k_slice(h, kc):
            base = N * (h // 2)
            return kT_sb[64 * (h % 2):64 * (h % 2) + 64,
                         base + 128 * kc:base + 128 * (kc + 1)]

        v3 = v1_sb[:].rearrange("p (kc h x) -> p kc h x", kc=NCHUNK, x=DV)

        # ---------------- pools ----------------
        main = ExitStack()
        wpool = main.enter_context(tc.tile_pool(name="wpool", bufs=4))
        P_pool = main.enter_context(tc.tile_pool(name="P_pool", bufs=25))
        dpool = main.enter_context(tc.tile_pool(name="dpool", bufs=2))
        ospool = main.enter_context(tc.tile_pool(name="ospool", bufs=3))
        spool = main.enter_context(
            tc.tile_pool(name="spool", bufs=2, space="PSUM"))   # 2x2 banks
        pvpool = main.enter_context(
            tc.tile_pool(name="pvpool", bufs=2, space="PSUM"))  # 2x1 banks
        gpool = main.enter_context(
            tc.tile_pool(name="gpool", bufs=2, space="PSUM"))   # 2x1 banks
        xq_stack = ExitStack()
        xqpool = xq_stack.enter_context(
            tc.tile_pool(name="xqpool", bufs=4, side="right"))
        xv_stack = ExitStack()
        xvpool = xv_stack.enter_context(
            tc.tile_pool(name="xvpool", bufs=2, side="right"))
        xk_stack = ExitStack()
        xkpool = xk_stack.enter_context(
            tc.tile_pool(name="xkpool", bufs=4, side="right"))

        # -------- input DMA: one batched transfer per half-tensor --------
        # DRAM [1024, n] viewed as [128 partitions, 8 cc-chunks, n].
        def dram3(t, lo, hi):
            return t[:].rearrange("(c p) n -> p c n", p=128)[:, :, lo:hi]

        wq_t = res.tile([128, 8 * CB], BF16, tag="wqt")
        wk_t = res.tile([128, 8 * CB], BF16, tag="wkt")
        wv_t = res.tile([128, 8 * CB], BF16, tag="wvt")
        wo_t = res.tile([128, 2 * C], BF16, tag="wot")
        wq3 = wq_t[:].rearrange("p (c n) -> p c n", c=8)
        wk3 = wk_t[:].rearrange("p (c n) -> p c n", c=8)
        wv3 = wv_t[:].rearrange("p (c n) -> p c n", c=8)
        wo3 = wo_t[:].rearrange("p (j n) -> p j n", j=2)

        xq_t = [xqpool.tile([128, 8 * 512], BF16, tag="xq", name=f"xq{i}")
                for i in range(4)]
        xk_t = [xkpool.tile([128, 8 * 512], BF16, tag="xk", name=f"xk{i}")
                for i in range(4)]
        xv_t = [xvpool.tile([128, 8 * 1024], BF16, tag="xv", name=f"xv{i}")
                for i in range(2)]
        xq3 = [t[:].rearrange("p (c n) -> p c n", c=8) for t in xq_t]
        xk3 = [t[:].rearrange("p (c n) -> p c n", c=8) for t in xk_t]
        xv3 = [t[:].rearrange("p (c n) -> p c n", c=8) for t in xv_t]

        def xq_sl(cc, qb):
            return xq3[qb][:, cc, :]

        def xk_sl(cc, kb):
            return xk3[kb][:, cc, :]

        nc.sync.dma_start(out=wq3[:], in_=dram3(wq, 0, CB))
        nc.scalar.dma_start(out=wk3[:], in_=dram3(wk, 0, CB))
        for i in range(4):
            nc.sync.dma_start(out=xq3[i][:], in_=dram3(xqT, 512 * i, 512 * (i + 1)))
            nc.scalar.dma_start(out=xk3[i][:], in_=dram3(xkT, 512 * i, 512 * (i + 1)))
        nc.sync.dma_start(out=wv3[:], in_=dram3(wv, 0, CB))
        for i in range(2):
            nc.sync.dma_start(out=xv3[i][:], in_=dram3(xvT, 1024 * i, 1024 * (i + 1)))
        nc.sync.dma_start(out=wo3[:],
                          in_=wo[:].rearrange("(j p) n -> p j n", p=128))

        nc.vector.memset(v3[:, :, :, D:DV], 1.0)

        # Warm the PE p-state during the input-DMA wait.  One long
        # ACCUMULATION group (start only on the first matmul) so the
        # junk matmuls stream without per-instruction semaphore chains.
        jk = gpool.tile([128, 512], F32, tag="g", name="junk")
        NJUNK = 36
        for i in range(NJUNK):
            nc.tensor.matmul(jk[:], v1_sb[:, 0:128], v1_sb[:, 0:512],
                             start=(i == 0), stop=(i == NJUNK - 1))

        # ---------------- building blocks ----------------
        P_tiles, PV, qk_state = {}, {}, {}

        def scores_pair(h, qb, pair):
            """S^T + exp for chunks (2*pair, 2*pair+1) of head h, qblock qb."""
            st = spool.tile([128, 1024], F32, tag="st", name=f"st_{h}_{qb}_{pair}")
            Pp = P_pool.tile([128, 1024], BF16, tag="P", name=f"P_{h}_{qb}_{pair}")
            for i in range(2):
                kc = 2 * pair + i
                nc.tensor.matmul(st[:, 512 * i:512 * (i + 1)],
                                 k_slice(h, kc), q_slice(h, qb),
                                 start=True, stop=True)
            nc.scalar.activation(Pp[:], st[:], Exp, scale=float(SCALE))
            P_tiles[(h, qb, pair)] = Pp

        def qk_proj_part(w3, x_sl, dst_sb, mb, qb, part, nparts):
            """1/nparts of one [128,512] Q^T/K^T projection group."""
            key = (id(w3), mb, qb)
            if part == 0:
                qk_state[key] = gpool.tile([128, 512], F32, tag="g",
                                           name=f"qk{mb}_{qb}_{id(w3) % 97}")
            ps = qk_state[key]
            step = 8 // nparts
            for cc in range(step * part, step * (part + 1)):
                nc.tensor.matmul(ps[:],
                                 w3[:, cc, 128 * mb:128 * (mb + 1)],
                                 x_sl(cc, qb),
                                 start=(cc == 0), stop=(cc == 7))
            if part == nparts - 1:
                nc.vector.tensor_copy(
                    dst_sb[:, N * mb + 512 * qb:N * mb + 512 * (qb + 1)], ps[:])
                del qk_state[key]

        def v_proj_block(tb):
            """V' for key-chunk tb: out[128 keys, 256] -> v1 cols 0:64."""
            ps = gpool.tile([128, 512], F32, tag="g", name=f"vps{tb}")
            for cc in range(8):
                nc.tensor.matmul(ps[:, 0:CB],
                                 xv3[tb // 8][:, cc, 128 * (tb % 8):
                                              128 * (tb % 8) + 128],
                                 wv3[:, cc, :],
                                 start=(cc == 0), stop=(cc == 7))
            nc.vector.tensor_copy(
                v3[:, tb, :, 0:D],
                ps[:, 0:CB].rearrange("p (h d) -> p h d", d=D))

        def pv_part(h, qb, pair):
            """Two PV chunk-matmuls for head h / qblock qb; finishes at pair 7.

            po rows 0:64 = O^T(h) raw, row 64 = softmax denominator.
            """
            if pair == 0:
                PV[(h, qb)] = pvpool.tile([128, 512], F32, tag="po",
                                          name=f"po{h}_{qb}")
            po = PV[(h, qb)]
            Pp = P_tiles.pop((h, qb, pair))
            for i in range(2):
                kc = 2 * pair + i
                nc.tensor.matmul(po[0:DV, :],
                                 v3[:, kc, h, :],
                                 Pp[:, 512 * i:512 * (i + 1)],
                                 start=(kc == 0), stop=(kc == NCHUNK - 1))
            if pair == 7:
                pv_finish(h, qb)

        def pv_finish(h, qb):
            """Normalize: A^T(h) = po[0:64] / po[64] -> aT plane."""
            po = PV.pop((h, qb))
            dinv = dpool.tile([64, 512], F32, tag="dinv", name=f"di{h}_{qb}")
            nc.vector.tensor_copy(draw_sb[:], po[64:65, :])
            nc.vector.reciprocal_approx_fast(drow_sb[:], draw_sb[:])
            nc.gpsimd.partition_broadcast(dinv[:], drow_sb[:])
            dst = aT0_sb if h < 2 else aT1_sb
            nc.vector.tensor_mul(
                dst[64 * (h % 2):64 * (h % 2) + 64, 512 * qb:512 * (qb + 1)],
                po[0:D, :], dinv[:])

        def oproj_m(qb, m, scalar_cast=False):
            """One m-block of the out-proj partial for query block qb.

            scalar_cast routes the psum->sbuf cast to ScalarE (a Copy,
            resident in every activation table set) -- used after the
            final exp, when ScalarE is idle, to unserialize the tail.
            """
            ps = gpool.tile([128, 512], F32, tag="g", name=f"ops{m}_{qb}")
            for j in range(2):
                aT = (aT0_sb, aT1_sb)[j]
                nc.tensor.matmul(ps[:], wo3[:, j, 128 * m:128 * (m + 1)],
                                 aT[:, 512 * qb:512 * (qb + 1)],
                                 start=(j == 0), stop=(j == 1))
            ev = ospool.tile([128, 512], BF16, tag="ev", name=f"oev{m}_{qb}")
            if scalar_cast:
                nc.scalar.copy(ev[:], ps[:])
            else:
                nc.vector.tensor_copy(ev[:], ps[:])
            nc.sync.dma_start(
                out=outT[128 * m:128 * (m + 1), 512 * qb:512 * (qb + 1)],
                in_=ev[:])

        # ---------------- emission ----------------
        # Pre-loop: Q^T(qb0) both planes + K^T plane-0 blocks 0/1 -- the
        # minimum for the first four score pairs.  K blocks 2/3 ride as
        # slot items ahead of the pairs that need them, so the first
        # exp is gated only by the first xq/xk quarters.
        for mb in range(2):
            qk_proj_part(wq3, xq_sl, qT_sb, mb, 0, 0, 1)
        for kb in range(2):
            qk_proj_part(wk3, xk_sl, kT_sb, 0, kb, 0, 1)

        # Slot schedule: one slot = one score group = one exp (~1.1us);
        # each slot carries <=~1.3us of extra PE work.
        #   qb0 h0: K^T plane-1 half-groups    (needed by h2 scores)
        #       h1: V' chunk 2p
        #       h2: V' chunk 2p+1 (pre), pv(h0) pair p
        #       h3: pv(h1) p, pv(h2) p-1, Q^T(qb1) quarters on p<4
        #       tail: pv(h2) pair 7        [pv(h3) rides in qb1's slots]
        #   qb>=1: pv(h-1) in-slot; fillers from the item list below;
        #       h3 additionally slot-lags pv(h3) by one pair, tail = pair 7.
        def run_qblock(qb, pre_items, post_items):
            lag = 2 if qb == 0 else 1
            for h in range(HC):
                for pair in range(8):
                    scores_pair(h, qb, pair)
                    for it in pre_items.get((h, pair), ()):
                        it()
                    if h >= lag:
                        pv_part(h - lag, qb, pair)
                    if h == HC - 1 and lag == 1 and pair >= 1:
                        pv_part(HC - 1, qb, pair - 1)
                    for it in post_items.get((h, pair), ()):
                        it()
            if qb == 0:
                pv_part(HC - 2, 0, 7)
            else:
                pv_part(HC - 1, qb, 7)

        pre0, post0 = {}, {}
        # K plane-0 blocks 2/3 ride the earliest h0 slots (well before
        # score pairs 4 and 6 need them); the displaced K plane-1
        # halves slide into h1 slots.
        post0[(0, 0)] = [lambda: qk_proj_part(wk3, xk_sl, kT_sb, 0, 2, 0, 1)]
        post0[(0, 1)] = [lambda: qk_proj_part(wk3, xk_sl, kT_sb, 0, 3, 0, 1)]
        km1 = [lambda kb=kb, part=part:
               qk_proj_part(wk3, xk_sl, kT_sb, 1, kb, part, 2)
               for kb in range(NQB) for part in range(2)]
        for p in (2, 3, 4, 5, 6, 7):
            post0[(0, p)] = [km1.pop(0)]
        for p in range(8):
            post0[(1, p)] = ([km1.pop(0)] if km1 else []) + \
                [lambda tb=2 * p: v_proj_block(tb)]
            pre0[(2, p)] = [lambda tb=2 * p + 1: v_proj_block(tb)]
            post0[(3, p)] = [lambda pp=p - 1: pv_part(2, 0, pp)] if p >= 1 else []
            if p < 4:
                post0[(3, p)] = post0.get((3, p), []) + \
                    [lambda mb=p // 2, part=p % 2:
                     qk_proj_part(wq3, xq_sl, qT_sb, mb, 1, part, 2)]
        run_qblock(0, pre0, post0)
        xk_stack.close()
        xv_stack.close()

        for qb in range(1, NQB):
            items = []
            if qb == 1:
                items += [lambda p=p: pv_part(3, 0, p) for p in range(8)]
            opq = [lambda m=m, q=qb - 1: oproj_m(q, m) for m in range(8)]
            if qb < NQB - 1:
                qqs = [lambda mb=mb, part=part, q=qb + 1:
                       qk_proj_part(wq3, xq_sl, qT_sb, mb, q, part, 4)
                       for mb in range(2) for part in range(4)]
                inter = [x for pair in zip(opq, qqs) for x in pair]
            else:
                inter = opq
            items += inter
            sched = {}
            for s, it in enumerate(items):
                sched[(s // 8, s % 8)] = sched.get((s // 8, s % 8), []) + [it]
            run_qblock(qb, {}, sched)
            if qb == NQB - 1:
                # keep the PE clock warm through the normalize chain so
                # the tail out-proj runs at full speed
                jk2 = gpool.tile([128, 512], F32, tag="g", name="junk2")
                for i in range(8):
                    nc.tensor.matmul(jk2[:], v1_sb[:, 0:128], v1_sb[:, 0:512],
                                     start=(i == 0), stop=(i == 7))
            if qb == NQB - 2:
                xq_stack.close()
        pv_finish(HC - 1, NQB - 1) if (HC - 1, NQB - 1) in PV else None
        for m in range(8):
            oproj_m(NQB - 1, m, scalar_cast=(m % 2 == 1))
        main.close()

    nc.compile()
    return nc


def _get_nc():
    if "nc" not in _CACHE:
        _CACHE["nc"] = build_nc()
    return _CACHE["nc"]


def _make_in_maps(q, k, v, Wq, Wk, Wv, Wo):
    bf = ml_dtypes.bfloat16
    q, k, v = np.asarray(q), np.asarray(k), np.asarray(v)
    qT = [np.ascontiguousarray(q[b].T).astype(bf) for b in range(B)]
    kT = [np.ascontiguousarray(k[b].T).astype(bf) for b in range(B)]
    vT = [np.ascontiguousarray(v[b].T).astype(bf) for b in range(B)]
    Wq, Wk, Wv, Wo = (np.asarray(x) for x in (Wq, Wk, Wv, Wo))
    wq_s = [np.ascontiguousarray(Wq[:, CB * g:CB * (g + 1)]).astype(bf)
            for g in range(4)]
    wk_s = [np.ascontiguousarray(Wk[:, CB * g:CB * (g + 1)]).astype(bf)
            for g in range(4)]
    wv_s = [np.ascontiguousarray(Wv[:, CB * g:CB * (g + 1)]).astype(bf)
            for g in range(4)]
    wo_s = [np.ascontiguousarray(Wo[CB * g:CB * (g + 1), :]).astype(bf)
            for g in range(4)]
    in_maps = []
    for c in range(8):
        b, g = c // 4, c % 4
        in_maps.append({
            "xqT": qT[b], "xkT": kT[b], "xvT": vT[b],
            "wq": wq_s[g], "wk": wk_s[g], "wv": wv_s[g], "wo": wo_s[g],
        })
    return in_maps


def _run(inputs, trace=False, **kw):
    nc = _get_nc()
    in_maps = _make_in_maps(inputs["q"], inputs["k"], inputs["v"],
                            inputs["Wq"], inputs["Wk"], inputs["Wv"], inputs["Wo"])
    res = None
    for attempt in range(3):
        try:
            res = run_bass_kernel_spmd(nc, in_maps, core_ids=list(range(8)),
                                       trace=trace, **kw)
            break
        except Exception:
            if attempt == 2:
                raise
            import time
            time.sleep(2.0)
    out = np.empty((B, N, C), np.float32)
    for b in range(B):
        acc = np.zeros((C, N), np.float32)
        for g in range(4):
            acc += res.results[4 * b + g]["outT"].astype(np.float32)
        out[b] = acc.T
    return out, res


def kernel(**inputs) -> np.ndarray:
    out, _ = _run(inputs, trace=False)
    return out
